# revision 8
# baseline (speedup 1.0000x reference)
"""Fused multi-head-attention (full-width variant) for 8 TRN2 NeuronCores.

Strategy: pure data-parallel over batch (B=8 -> one batch per core).
Per core, with everything in "feature-on-partition" transposed layouts:
  kT/qT = Wk/Wq @ xT               (fp16 matmuls, fp32 PSUM; the x8 energy
                                    scale is folded into Wq on host; fp16
                                    keeps the tf32-grade 10-bit mantissa but
                                    streams at bf16 speed — f32r moving
                                    operands cost ~7% more per matmul)
  vT    = Wv @ xT                  (fp16 matmuls, stored bf16)
  vo    = vT.T @ Wo.T              (bf16; folds the out-projection into V:
                                    y = P @ (v @ Wo.T), so attention's PV
                                    matmul directly produces yT)
  E     = q @ k.T                  (fp16, fp32 PSUM accum; already x8)
  P     = softmax rows via ACT exp (bias=-rowmax via negated reduce)
  PT    = DMA-xbar transpose of P  (bf16)
  yT    = vo.T @ PT                (bf16 matmuls, 512-wide)
Host transposes x/W in (casting to fp16), yT out.

The tensor queue executes in static program order, so blocks are
software-pipelined at emission time:
  E(b0), vo-proj, E(b1), PV(b0), E(b2), PV(b1), E(b3), PV(b2), PV(b3)
PV(ib) needs block ib's four P-transposes (ready ~7us after E(ib)
ends); the interposed E(ib+1) covers that latency so the PE never
stalls. The wo load is dispatched right after the xT pool frees so it
lands during E(b0), before vo-proj heads the queue.

DMA queues: transposes + their guard DMAs + input loads on the sync
(SP) HWDGE queue (inputs in dependency order, xT-nb0 split in three
e-chunk pairs so the first kT matmul can start earliest); output
stores on the scalar (Activation) HWDGE queue so they never delay a
transpose dispatch.
"""
import sys

sys.path.insert(0, "/opt/trn_rl_repo")

import numpy as np

import concourse.bass as bass  # noqa: F401
import concourse.tile as tile
from concourse import bacc, mybir

F32 = mybir.dt.float32
F16 = mybir.dt.float16
BF16 = mybir.dt.bfloat16
AX = mybir.AxisListType.X
MAX = mybir.AluOpType.max

B = 8
E = 768
N = 2048
EC = E // 128      # 6 feature chunks
NT = N // 128      # 16 token chunks
NBLK = N // 512    # 4 blocks of 512 tokens
SCALE = 8.0        # sqrt(head_dim); reference multiplies by it

_CACHE = {}


def _build():
    nc = bacc.Bacc("TRN2", target_bir_lowering=False, debug=False, num_devices=B)

    xT_d = nc.dram_tensor("xT", [E, N], F16, kind="ExternalInput")
    wq_d = nc.dram_tensor("wq", [E, E], F16, kind="ExternalInput")
    wk_d = nc.dram_tensor("wk", [E, E], F16, kind="ExternalInput")
    wv_d = nc.dram_tensor("wv", [E, E], F16, kind="ExternalInput")
    wo_d = nc.dram_tensor("wo", [E, E], BF16, kind="ExternalInput")
    yT_d = nc.dram_tensor("yT", [E, N], F16, kind="ExternalOutput")
    # Tiny per-tile stats dump. Its real job: a plain HWDGE DMA queued before
    # every dma_start_transpose — two xbar transposes back-to-back on the sync
    # queue with no intervening plain DMA produce doubled output values
    # (observed on HW; the plain transfer forces the xbar-mode transition).
    snk_d = nc.dram_tensor("snk", [NT, 128, 1], F32, kind="ExternalOutput")

    xT_r = xT_d.rearrange("(c p) n -> p c n", p=128)
    wq_r = wq_d.rearrange("(c p) f -> p c f", p=128)
    wk_r = wk_d.rearrange("(c p) f -> p c f", p=128)
    wv_r = wv_d.rearrange("(c p) f -> p c f", p=128)
    wo_r = wo_d.rearrange("(c p) f -> p c f", p=128)
    yT_r = yT_d.rearrange("(c p) n -> p c n", p=128)

    with tile.TileContext(nc) as tc:
        with tc.tile_pool(name="kT", bufs=1) as ktp, \
             tc.tile_pool(name="qT", bufs=1) as qtp, \
             tc.tile_pool(name="vT", bufs=1) as vtp, \
             tc.tile_pool(name="pse", bufs=6, space="PSUM") as pse, \
             tc.tile_pool(name="psm", bufs=2, space="PSUM") as psm:
            kT = ktp.tile([128, EC, N], F16)    # 24 KB/partition
            qT = qtp.tile([128, EC, N], F16)    # 24
            vT = vtp.tile([128, EC, N], BF16)   # 24

            # ---------------- stage B: projections ----------------
            with tc.tile_pool(name="xt", bufs=1) as xtp, \
                 tc.tile_pool(name="wp", bufs=2) as wpp:
                # PE warm-up during the initial input-DMA window: dummy
                # matmuls push the HAM activity window so the first real
                # matmuls run at 2.4 GHz instead of 1.2 GHz
                wrm = xtp.tile([128, 512], BF16, tag="wrm")
                nc.vector.memset(wrm[:], 0.0)
                for _w in range(10):
                    wps = pse.tile([128, 512], F32, tag="ps")
                    nc.tensor.matmul(
                        wps[:],
                        lhsT=wrm[:, 0:128],
                        rhs=wrm[:],
                        start=True,
                        stop=True,
                    )
                xT = xtp.tile([128, EC, N], F16)  # 24
                wk_t = wpp.tile([128, EC, E], F16, tag="w")  # 9 x2
                # DMA order tuned for earliest sustained PE start: wk-f0 and
                # the first e-chunks of xT-nb0 first (minimum for the kT
                # f0/nb0 accumulation), then the rest of wk, then remaining
                # xT blocks, wq, wv
                nc.sync.dma_start(wk_t[:, :, 0:128], wk_r[:, :, 0:128])
                nc.sync.dma_start(xT[:, 0:2, 0:512], xT_r[:, 0:2, 0:512])
                nc.sync.dma_start(xT[:, 2:4, 0:512], xT_r[:, 2:4, 0:512])
                nc.sync.dma_start(xT[:, 4:6, 0:512], xT_r[:, 4:6, 0:512])
                for f in range(1, EC):
                    nc.sync.dma_start(
                        wk_t[:, :, f * 128:(f + 1) * 128],
                        wk_r[:, :, f * 128:(f + 1) * 128],
                    )
                nc.sync.dma_start(xT[:, :, 512:1024], xT_r[:, :, 512:1024])
                nc.sync.dma_start(xT[:, :, 1024:1536], xT_r[:, :, 1024:1536])
                nc.sync.dma_start(xT[:, :, 1536:2048], xT_r[:, :, 1536:2048])
                wq_t = wpp.tile([128, EC, E], F16, tag="w")
                nc.sync.dma_start(wq_t[:], wq_r[:])

                def proj(dst, w_t):
                    # dst = W @ xT   (nb-outer: group nb needs only xT blk nb)
                    for nb in range(NBLK):
                        for f in range(EC):
                            ps = pse.tile([128, 512], F32, tag="ps")
                            for e in range(EC):
                                nc.tensor.matmul(
                                    ps[:],
                                    lhsT=w_t[:, e, f * 128:(f + 1) * 128],
                                    rhs=xT[:, e, nb * 512:(nb + 1) * 512],
                                    start=(e == 0),
                                    stop=(e == EC - 1),
                                )
                            nc.vector.tensor_copy(
                                dst[:, f, nb * 512:(nb + 1) * 512], ps[:]
                            )

                proj(kT, wk_t)
                proj(qT, wq_t)
                # vT stored bf16; wv reuses wk's slot
                wv_t = wpp.tile([128, EC, E], F16, tag="w")
                nc.sync.dma_start(wv_t[:], wv_r[:])
                proj(vT, wv_t)

            # ---------------- attention + vo, software-pipelined ----------
            with tc.tile_pool(name="vo", bufs=1) as vop, \
                 tc.tile_pool(name="wo", bufs=1) as wop, \
                 tc.tile_pool(name="pt", bufs=2) as ptp, \
                 tc.tile_pool(name="pp", bufs=4) as ppp, \
                 tc.tile_pool(name="yt", bufs=2) as ytp, \
                 tc.tile_pool(name="st", bufs=8) as stp:
                vo = vop.tile([128, NT, E], BF16)   # 24; [j-part, jc, f]
                wo_t = wop.tile([128, EC, E], BF16)  # 9
                # dispatched on sync after the inputs; its SBUF region
                # overlaps freed xT, so it lands right after vT-proj ends,
                # during E(b0) — before vo-proj heads the tensor queue
                nc.sync.dma_start(wo_t[:], wo_r[:])

                pt_blks = [None] * NBLK

                def energy_block(ib):
                    pt_blk = ptp.tile([128, NT, 512], BF16)  # 16 x2
                    pt_blks[ib] = pt_blk
                    for t4 in range(4):
                        i = ib * 4 + t4
                        stats = stp.tile([128, 8], F32, tag="stats")
                        e_tiles = []
                        for jb in range(NBLK):
                            pe = pse.tile([128, 512], F32, tag="ps")
                            for d in range(EC):
                                nc.tensor.matmul(
                                    pe[:],
                                    lhsT=qT[:, d, i * 128:(i + 1) * 128],
                                    rhs=kT[:, d, jb * 512:(jb + 1) * 512],
                                    start=(d == 0),
                                    stop=(d == EC - 1),
                                )
                            nc.vector.tensor_reduce(
                                stats[:, jb:jb + 1], pe[:], axis=AX, op=MAX
                            )
                            e_tiles.append(pe)
                        nmax = stp.tile([128, 1], F32, tag="nmax")
                        nc.vector.tensor_reduce(
                            nmax[:], stats[:, 0:4], axis=AX, op=MAX,
                            negate=True,
                        )
                        p_t = ppp.tile([128, N], BF16)  # 4 x4
                        for jb in range(NBLK):
                            nc.scalar.activation(
                                p_t[:, jb * 512:(jb + 1) * 512],
                                e_tiles[jb][:],
                                func=mybir.ActivationFunctionType.Exp,
                                bias=nmax[:],
                                scale=1.0,
                                accum_out=stats[:, 4 + jb:5 + jb],
                            )
                        rs = stp.tile([128, 1], F32, tag="rs")
                        nc.vector.tensor_reduce(
                            rs[:], stats[:, 4:8], axis=AX,
                            op=mybir.AluOpType.add,
                        )
                        rcp = stp.tile([128, 1], F32, tag="rcp")
                        nc.vector.reciprocal(rcp[:], rs[:])
                        nc.vector.tensor_scalar_mul(p_t[:], p_t[:], rcp[:])
                        # ALL transposes on one HWDGE queue, each preceded by
                        # a plain guard DMA: concurrent xbar transposes (even
                        # on different queues) corrupt results — xbar state
                        # is per-core global
                        nc.sync.dma_start(snk_d[i], stats[:, 0:1])
                        nc.sync.dma_start_transpose(
                            pt_blk[:, :, t4 * 128:(t4 + 1) * 128], p_t[:]
                        )

                def pv_block(ib):
                    # yT-block = vo.T @ PT (bf16, 512-wide). The very last
                    # f of the last block is split in halves so the final
                    # store starts ~1.7us earlier.
                    pt_blk = pt_blks[ib]
                    for f in range(EC):
                        last = (ib == NBLK - 1 and f == EC - 1)
                        halves = ((0, 256), (256, 512)) if last else ((0, 512),)
                        for lo, hi in halves:
                            py = psm.tile([128, 512], F32, tag="mm")
                            for jc in range(NT):
                                nc.tensor.matmul(
                                    py[:, :hi - lo],
                                    lhsT=vo[:, jc, f * 128:(f + 1) * 128],
                                    rhs=pt_blk[:, jc, lo:hi],
                                    start=(jc == 0),
                                    stop=(jc == NT - 1),
                                )
                            yt = ytp.tile([128, 512], F16)
                            nc.vector.tensor_copy(
                                yt[:, :hi - lo], py[:, :hi - lo]
                            )
                            nc.scalar.dma_start(
                                yT_r[:, f, ib * 512 + lo:ib * 512 + hi],
                                yt[:, :hi - lo],
                            )

                energy_block(0)

                # vo = vT.T @ Wo.T (bf16); runs while E(b0)'s softmax chain
                # and transposes drain
                for jc in range(NT):
                    for flo, fhi in ((0, 512), (512, 768)):
                        ps = psm.tile([128, 512], F32, tag="mm")
                        for c in range(EC):
                            nc.tensor.matmul(
                                ps[:, :fhi - flo],
                                lhsT=vT[:, c, jc * 128:(jc + 1) * 128],
                                rhs=wo_t[:, c, flo:fhi],
                                start=(c == 0),
                                stop=(c == EC - 1),
                            )
                        nc.vector.tensor_copy(vo[:, jc, flo:fhi], ps[:, :fhi - flo])

                energy_block(1)
                pv_block(0)
                energy_block(2)
                pv_block(1)
                energy_block(3)
                pv_block(2)
                pv_block(3)

    nc.finalize()
    return nc


def _get_nc():
    if "nc" not in _CACHE:
        _CACHE["nc"] = _build()
    return _CACHE["nc"]


def kernel(x, Wq, Wk, Wv, Wo, _run_kwargs=None):
    import ml_dtypes
    from concourse.bass_utils import run_bass_kernel_spmd

    x = np.asarray(x, dtype=np.float32)
    # fold the sqrt(head_dim) energy scale into Wq (exact: power of 2)
    wq = np.ascontiguousarray(
        (np.asarray(Wq, dtype=np.float32).T * SCALE).astype(np.float16)
    )
    wk = np.ascontiguousarray(np.asarray(Wk, dtype=np.float32).T.astype(np.float16))
    wv = np.ascontiguousarray(np.asarray(Wv, dtype=np.float32).T.astype(np.float16))
    wo = np.ascontiguousarray(
        np.asarray(Wo, dtype=np.float32).T.astype(ml_dtypes.bfloat16)
    )

    nc = _get_nc()
    in_maps = [
        {
            "xT": np.ascontiguousarray(x[b].T.astype(np.float16)),
            "wq": wq,
            "wk": wk,
            "wv": wv,
            "wo": wo,
        }
        for b in range(B)
    ]
    res = run_bass_kernel_spmd(nc, in_maps, list(range(B)), **(_run_kwargs or {}))
    out = np.stack([res.results[b]["yT"].T for b in range(B)])
    if _run_kwargs:
        _CACHE["last_results"] = res
    return np.ascontiguousarray(out, dtype=np.float32)


# revision 11
# speedup vs baseline: 1.0744x; 1.0744x over previous
"""Fused multi-head-attention (full-width variant) for 8 TRN2 NeuronCores.

Strategy: pure data-parallel over batch (B=8 -> one batch per core).
Per core, with everything in "feature-on-partition" transposed layouts:
  kT/qT = Wk/Wq @ xT               (fp16 matmuls, fp32 PSUM; the x8 energy
                                    scale is folded into Wq on host; fp16
                                    keeps the tf32-grade 10-bit mantissa but
                                    streams at bf16 speed — f32r moving
                                    operands cost ~7% more per matmul)
  vT    = Wv @ xT                  (fp16 matmuls, stored bf16)
  vo    = vT.T @ Wo.T              (bf16; folds the out-projection into V:
                                    y = P @ (v @ Wo.T), so attention's PV
                                    matmul directly produces yT)
  E     = q @ k.T                  (fp16, fp32 PSUM accum; already x8)
  P     = softmax rows via ACT exp (bias=-rowmax via negated reduce)
  PT    = DMA-xbar transpose of P  (bf16)
  yT    = vo.T @ PT                (bf16 matmuls, 512-wide)
Host transposes x/W in (casting to fp16), yT out.

The tensor queue executes in static program order, so blocks are
software-pipelined at emission time:
  E(b0), vo-proj, E(b1), PV(b0), E(b2), PV(b1), E(b3), PV(b2), PV(b3)
PV(ib) needs block ib's four P-transposes (ready ~7us after E(ib)
ends); the interposed E(ib+1) covers that latency so the PE never
stalls. The wo load is dispatched right after the xT pool frees so it
lands during E(b0), before vo-proj heads the queue.

DMA queues: transposes + their guard DMAs + input loads on the sync
(SP) HWDGE queue (inputs in dependency order, xT-nb0 split in three
e-chunk pairs so the first kT matmul can start earliest); output
stores on the scalar (Activation) HWDGE queue so they never delay a
transpose dispatch.
"""
import sys

sys.path.insert(0, "/opt/trn_rl_repo")

import numpy as np

import concourse.bass as bass  # noqa: F401
import concourse.tile as tile
from concourse import bacc, mybir

F32 = mybir.dt.float32
F16 = mybir.dt.float16
BF16 = mybir.dt.bfloat16
AX = mybir.AxisListType.X
MAX = mybir.AluOpType.max

B = 8
E = 768
N = 2048
EC = E // 128      # 6 feature chunks
NT = N // 128      # 16 token chunks
NBLK = N // 512    # 4 blocks of 512 tokens
SCALE = 8.0        # sqrt(head_dim); reference multiplies by it

_CACHE = {}


def _build():
    nc = bacc.Bacc("TRN2", target_bir_lowering=False, debug=False, num_devices=B)

    xT_d = nc.dram_tensor("xT", [E, N], F16, kind="ExternalInput")
    wq_d = nc.dram_tensor("wq", [E, E], F16, kind="ExternalInput")
    wk_d = nc.dram_tensor("wk", [E, E], F16, kind="ExternalInput")
    wv_d = nc.dram_tensor("wv", [E, E], F16, kind="ExternalInput")
    wo_d = nc.dram_tensor("wo", [E, E], BF16, kind="ExternalInput")
    yT_d = nc.dram_tensor("yT", [E, N], F16, kind="ExternalOutput")
    # Tiny per-tile stats dump. Its real job: a plain HWDGE DMA queued before
    # every dma_start_transpose — two xbar transposes back-to-back on the sync
    # queue with no intervening plain DMA produce doubled output values
    # (observed on HW; the plain transfer forces the xbar-mode transition).
    snk_d = nc.dram_tensor("snk", [NT, 128, 1], F32, kind="ExternalOutput")

    xT_r = xT_d.rearrange("(c p) n -> p c n", p=128)
    wq_r = wq_d.rearrange("(c p) f -> p c f", p=128)
    wk_r = wk_d.rearrange("(c p) f -> p c f", p=128)
    wv_r = wv_d.rearrange("(c p) f -> p c f", p=128)
    wo_r = wo_d.rearrange("(c p) f -> p c f", p=128)
    yT_r = yT_d.rearrange("(c p) n -> p c n", p=128)

    with tile.TileContext(nc) as tc:
        with tc.tile_pool(name="kT", bufs=1) as ktp, \
             tc.tile_pool(name="qT", bufs=1) as qtp, \
             tc.tile_pool(name="vT", bufs=1) as vtp, \
             tc.tile_pool(name="pse", bufs=6, space="PSUM") as pse, \
             tc.tile_pool(name="psm", bufs=2, space="PSUM") as psm:
            kT = ktp.tile([128, EC, N], F16)    # 24 KB/partition
            qT = qtp.tile([128, EC, N], F16)    # 24
            vT = vtp.tile([128, EC, N], BF16)   # 24

            # ---------------- stage B: projections ----------------
            with tc.tile_pool(name="xt", bufs=1) as xtp, \
                 tc.tile_pool(name="wp", bufs=2) as wpp:
                # PE warm-up during the initial input-DMA window: dummy
                # matmuls push the HAM activity window so the first real
                # matmuls run at 2.4 GHz instead of 1.2 GHz
                wrm = xtp.tile([128, 512], BF16, tag="wrm")
                nc.vector.memset(wrm[:], 0.0)
                for _w in range(10):
                    wps = pse.tile([128, 512], F32, tag="ps")
                    nc.tensor.matmul(
                        wps[:],
                        lhsT=wrm[:, 0:128],
                        rhs=wrm[:],
                        start=True,
                        stop=True,
                    )
                xT = xtp.tile([128, EC, N], F16)  # 24
                wk_t = wpp.tile([128, EC, E], F16, tag="w")  # 9 x2
                # DMA order tuned for earliest sustained PE start: wk-f0 and
                # the first e-chunks of xT-nb0 first (minimum for the kT
                # f0/nb0 accumulation), then the rest of wk, then remaining
                # xT blocks, wq, wv
                nc.sync.dma_start(wk_t[:, :, 0:128], wk_r[:, :, 0:128])
                nc.sync.dma_start(xT[:, 0:2, 0:512], xT_r[:, 0:2, 0:512])
                nc.sync.dma_start(xT[:, 2:4, 0:512], xT_r[:, 2:4, 0:512])
                nc.sync.dma_start(xT[:, 4:6, 0:512], xT_r[:, 4:6, 0:512])
                for f in range(1, EC):
                    nc.sync.dma_start(
                        wk_t[:, :, f * 128:(f + 1) * 128],
                        wk_r[:, :, f * 128:(f + 1) * 128],
                    )
                nc.sync.dma_start(xT[:, :, 512:1024], xT_r[:, :, 512:1024])
                nc.sync.dma_start(xT[:, :, 1024:1536], xT_r[:, :, 1024:1536])
                nc.sync.dma_start(xT[:, :, 1536:2048], xT_r[:, :, 1536:2048])
                wq_t = wpp.tile([128, EC, E], F16, tag="w")
                nc.sync.dma_start(wq_t[:], wq_r[:])

                def proj(dst, w_t):
                    # dst = W @ xT   (nb-outer: group nb needs only xT blk nb)
                    for nb in range(NBLK):
                        for f in range(EC):
                            ps = pse.tile([128, 512], F32, tag="ps")
                            for e in range(EC):
                                nc.tensor.matmul(
                                    ps[:],
                                    lhsT=w_t[:, e, f * 128:(f + 1) * 128],
                                    rhs=xT[:, e, nb * 512:(nb + 1) * 512],
                                    start=(e == 0),
                                    stop=(e == EC - 1),
                                )
                            nc.vector.tensor_copy(
                                dst[:, f, nb * 512:(nb + 1) * 512], ps[:]
                            )

                proj(kT, wk_t)
                proj(qT, wq_t)
                # vT stored bf16; wv reuses wk's slot
                wv_t = wpp.tile([128, EC, E], F16, tag="w")
                nc.sync.dma_start(wv_t[:], wv_r[:])
                proj(vT, wv_t)

            # ---------------- attention + vo, software-pipelined ----------
            with tc.tile_pool(name="vo", bufs=1) as vop, \
                 tc.tile_pool(name="wo", bufs=1) as wop, \
                 tc.tile_pool(name="pt", bufs=2) as ptp, \
                 tc.tile_pool(name="pp", bufs=4) as ppp, \
                 tc.tile_pool(name="yt", bufs=12) as ytp, \
                 tc.tile_pool(name="st", bufs=8) as stp:
                vo = vop.tile([128, NT, E], BF16)   # 24; [j-part, jc, f]
                wo_t = wop.tile([128, EC, E], BF16)  # 9
                # dispatched on sync after the inputs; its SBUF region
                # overlaps freed xT, so it lands right after vT-proj ends,
                # during E(b0) — before vo-proj heads the tensor queue
                nc.sync.dma_start(wo_t[:], wo_r[:])

                pt_blks = [None] * NBLK

                def energy_block(ib):
                    pt_blk = ptp.tile([128, NT, 512], BF16)  # 16 x2
                    pt_blks[ib] = pt_blk
                    for t4 in range(4):
                        i = ib * 4 + t4
                        stats = stp.tile([128, 8], F32, tag="stats")
                        e_tiles = []
                        for jb in range(NBLK):
                            pe = pse.tile([128, 512], F32, tag="ps")
                            for d in range(EC):
                                nc.tensor.matmul(
                                    pe[:],
                                    lhsT=qT[:, d, i * 128:(i + 1) * 128],
                                    rhs=kT[:, d, jb * 512:(jb + 1) * 512],
                                    start=(d == 0),
                                    stop=(d == EC - 1),
                                )
                            nc.vector.tensor_reduce(
                                stats[:, jb:jb + 1], pe[:], axis=AX, op=MAX
                            )
                            e_tiles.append(pe)
                        nmax = stp.tile([128, 1], F32, tag="nmax")
                        nc.vector.tensor_reduce(
                            nmax[:], stats[:, 0:4], axis=AX, op=MAX,
                            negate=True,
                        )
                        p_t = ppp.tile([128, N], BF16)  # 4 x4
                        for jb in range(NBLK):
                            nc.scalar.activation(
                                p_t[:, jb * 512:(jb + 1) * 512],
                                e_tiles[jb][:],
                                func=mybir.ActivationFunctionType.Exp,
                                bias=nmax[:],
                                scale=1.0,
                                accum_out=stats[:, 4 + jb:5 + jb],
                            )
                        rs = stp.tile([128, 1], F32, tag="rs")
                        nc.vector.tensor_reduce(
                            rs[:], stats[:, 4:8], axis=AX,
                            op=mybir.AluOpType.add,
                        )
                        rcp = stp.tile([128, 1], F32, tag="rcp")
                        nc.vector.reciprocal(rcp[:], rs[:])
                        nc.vector.tensor_scalar_mul(p_t[:], p_t[:], rcp[:])
                        # ALL transposes on one HWDGE queue, each preceded by
                        # a plain guard DMA: concurrent xbar transposes (even
                        # on different queues) corrupt results — xbar state
                        # is per-core global
                        nc.sync.dma_start(snk_d[i], stats[:, 0:1])
                        nc.sync.dma_start_transpose(
                            pt_blk[:, :, t4 * 128:(t4 + 1) * 128], p_t[:]
                        )

                yts = [[] for _ in range(NBLK)]

                def pv_block(ib):
                    # yT-block = vo.T @ PT (bf16, 512-wide). The very last
                    # f of the last block is split in halves so the final
                    # store starts ~1.7us earlier. Stores are NOT emitted
                    # here — see store_block.
                    pt_blk = pt_blks[ib]
                    for f in range(EC):
                        last = (ib == NBLK - 1 and f == EC - 1)
                        halves = ((0, 256), (256, 512)) if last else ((0, 512),)
                        for lo, hi in halves:
                            py = psm.tile([128, 512], F32, tag="mm")
                            for jc in range(NT):
                                nc.tensor.matmul(
                                    py[:, :hi - lo],
                                    lhsT=vo[:, jc, f * 128:(f + 1) * 128],
                                    rhs=pt_blk[:, jc, lo:hi],
                                    start=(jc == 0),
                                    stop=(jc == NT - 1),
                                )
                            yt = ytp.tile([128, 512], F16)
                            nc.vector.tensor_copy(
                                yt[:, :hi - lo], py[:, :hi - lo]
                            )
                            yts[ib].append((yt, f, lo, hi))

                def store_block(ib):
                    # yT stores dispatch from the scalar queue. Emitted one
                    # block AFTER the matching pv_block: a store whose yt
                    # evac is pending head-of-line-blocks the next exps on
                    # the scalar queue, which stalls the transposes the PV
                    # matmuls need — a four-engine cycle. One block of
                    # separation guarantees the yt data is ready before any
                    # exp queues behind the store.
                    for yt, f, lo, hi in yts[ib]:
                        nc.scalar.dma_start(
                            yT_r[:, f, ib * 512 + lo:ib * 512 + hi],
                            yt[:, :hi - lo],
                        )

                energy_block(0)

                # vo = vT.T @ Wo.T (bf16); runs while E(b0)'s softmax chain
                # and transposes drain
                for jc in range(NT):
                    for flo, fhi in ((0, 512), (512, 768)):
                        ps = psm.tile([128, 512], F32, tag="mm")
                        for c in range(EC):
                            nc.tensor.matmul(
                                ps[:, :fhi - flo],
                                lhsT=vT[:, c, jc * 128:(jc + 1) * 128],
                                rhs=wo_t[:, c, flo:fhi],
                                start=(c == 0),
                                stop=(c == EC - 1),
                            )
                        nc.vector.tensor_copy(vo[:, jc, flo:fhi], ps[:, :fhi - flo])

                energy_block(1)
                pv_block(0)
                energy_block(2)
                store_block(0)
                pv_block(1)
                energy_block(3)
                store_block(1)
                pv_block(2)
                store_block(2)
                pv_block(3)
                store_block(3)

    nc.finalize()
    return nc


def _get_nc():
    if "nc" not in _CACHE:
        _CACHE["nc"] = _build()
    return _CACHE["nc"]


def kernel(x, Wq, Wk, Wv, Wo, _run_kwargs=None):
    import ml_dtypes
    from concourse.bass_utils import run_bass_kernel_spmd

    x = np.asarray(x, dtype=np.float32)
    # fold the sqrt(head_dim) energy scale into Wq (exact: power of 2)
    wq = np.ascontiguousarray(
        (np.asarray(Wq, dtype=np.float32).T * SCALE).astype(np.float16)
    )
    wk = np.ascontiguousarray(np.asarray(Wk, dtype=np.float32).T.astype(np.float16))
    wv = np.ascontiguousarray(np.asarray(Wv, dtype=np.float32).T.astype(np.float16))
    wo = np.ascontiguousarray(
        np.asarray(Wo, dtype=np.float32).T.astype(ml_dtypes.bfloat16)
    )

    nc = _get_nc()
    in_maps = [
        {
            "xT": np.ascontiguousarray(x[b].T.astype(np.float16)),
            "wq": wq,
            "wk": wk,
            "wv": wv,
            "wo": wo,
        }
        for b in range(B)
    ]
    res = run_bass_kernel_spmd(nc, in_maps, list(range(B)), **(_run_kwargs or {}))
    out = np.stack([res.results[b]["yT"].T for b in range(B)])
    if _run_kwargs:
        _CACHE["last_results"] = res
    return np.ascontiguousarray(out, dtype=np.float32)


# revision 12
# speedup vs baseline: 1.1476x; 1.0682x over previous
"""Fused multi-head-attention (full-width variant) for 8 TRN2 NeuronCores.

Strategy: pure data-parallel over batch (B=8 -> one batch per core).
Per core, with everything in "feature-on-partition" transposed layouts:
  kT/qT = Wk/Wq @ xT               (fp16 matmuls, fp32 PSUM; the x8 energy
                                    scale is folded into Wq on host; fp16
                                    keeps the tf32-grade 10-bit mantissa but
                                    streams at bf16 speed — f32r moving
                                    operands cost ~7% more per matmul)
  vT    = Wv @ xT                  (fp16 matmuls, stored bf16)
  vo    = vT.T @ Wo.T              (bf16; folds the out-projection into V:
                                    y = P @ (v @ Wo.T), so attention's PV
                                    matmul directly produces yT)
  E     = q @ k.T                  (fp16, fp32 PSUM accum; already x8)
  P     = softmax rows via ACT exp (bias=-rowmax via negated reduce)
  PT    = DMA-xbar transpose of P  (bf16)
  yT    = vo.T @ PT                (bf16 matmuls, 512-wide)
Host transposes x/W in (casting to fp16), yT out.

The tensor queue executes in static program order, so blocks are
software-pipelined at emission time:
  E(b0), vo-proj, E(b1), PV(b0), E(b2), PV(b1), E(b3), PV(b2), PV(b3)
PV(ib) needs block ib's four P-transposes (ready ~7us after E(ib)
ends); the interposed E(ib+1) covers that latency so the PE never
stalls. The wo load is dispatched right after the xT pool frees so it
lands during E(b0), before vo-proj heads the queue.

DMA queues: transposes + their guard DMAs + input loads on the sync
(SP) HWDGE queue (inputs in dependency order, xT-nb0 split in three
e-chunk pairs so the first kT matmul can start earliest); output
stores on the scalar (Activation) HWDGE queue so they never delay a
transpose dispatch.
"""
import sys

sys.path.insert(0, "/opt/trn_rl_repo")

import numpy as np

import concourse.bass as bass  # noqa: F401
import concourse.tile as tile
from concourse import bacc, mybir

F32 = mybir.dt.float32
F16 = mybir.dt.float16
BF16 = mybir.dt.bfloat16
AX = mybir.AxisListType.X
MAX = mybir.AluOpType.max

B = 8
E = 768
N = 2048
EC = E // 128      # 6 feature chunks
NT = N // 128      # 16 token chunks
NBLK = N // 512    # 4 blocks of 512 tokens
SCALE = 8.0        # sqrt(head_dim); reference multiplies by it

_CACHE = {}


def _build():
    nc = bacc.Bacc("TRN2", target_bir_lowering=False, debug=False, num_devices=B)

    xT_d = nc.dram_tensor("xT", [E, N], F16, kind="ExternalInput")
    wq_d = nc.dram_tensor("wq", [E, E], F16, kind="ExternalInput")
    wk_d = nc.dram_tensor("wk", [E, E], F16, kind="ExternalInput")
    wv_d = nc.dram_tensor("wv", [E, E], F16, kind="ExternalInput")
    wo_d = nc.dram_tensor("wo", [E, E], BF16, kind="ExternalInput")
    yT_d = nc.dram_tensor("yT", [E, N], F16, kind="ExternalOutput")
    # Tiny per-tile stats dump. Its real job: a plain HWDGE DMA queued before
    # every dma_start_transpose — two xbar transposes back-to-back on the sync
    # queue with no intervening plain DMA produce doubled output values
    # (observed on HW; the plain transfer forces the xbar-mode transition).
    snk_d = nc.dram_tensor("snk", [NT, 128, 8], F32, kind="ExternalOutput")

    xT_r = xT_d.rearrange("(c p) n -> p c n", p=128)
    wq_r = wq_d.rearrange("(c p) f -> p c f", p=128)
    wk_r = wk_d.rearrange("(c p) f -> p c f", p=128)
    wv_r = wv_d.rearrange("(c p) f -> p c f", p=128)
    wo_r = wo_d.rearrange("(c p) f -> p c f", p=128)
    yT_r = yT_d.rearrange("(c p) n -> p c n", p=128)

    with tile.TileContext(nc) as tc:
        with tc.tile_pool(name="kT", bufs=1) as ktp, \
             tc.tile_pool(name="qT", bufs=1) as qtp, \
             tc.tile_pool(name="vT", bufs=1) as vtp, \
             tc.tile_pool(name="pse", bufs=6, space="PSUM") as pse, \
             tc.tile_pool(name="psm", bufs=2, space="PSUM") as psm:
            kT = ktp.tile([128, EC, N], F16)    # 24 KB/partition
            qT = qtp.tile([128, EC, N], F16)    # 24
            vT = vtp.tile([128, EC, N], BF16)   # 24

            # ---------------- stage B: projections ----------------
            with tc.tile_pool(name="xt", bufs=1) as xtp, \
                 tc.tile_pool(name="wp", bufs=2) as wpp:
                # PE warm-up during the initial input-DMA window: dummy
                # matmuls push the HAM activity window so the first real
                # matmuls run at 2.4 GHz instead of 1.2 GHz
                wrm = xtp.tile([128, 512], BF16, tag="wrm")
                nc.vector.memset(wrm[:], 0.0)
                for _w in range(8):
                    wps = pse.tile([128, 512], F32, tag="ps")
                    nc.tensor.matmul(
                        wps[:],
                        lhsT=wrm[:, 0:128],
                        rhs=wrm[:],
                        start=True,
                        stop=True,
                    )
                xT = xtp.tile([128, EC, N], F16)  # 24
                wk_t = wpp.tile([128, EC, E], F16, tag="w")  # 9 x2
                # DMA order tuned for earliest sustained PE start: wk-f0 and
                # the first e-chunks of xT-nb0 first (minimum for the kT
                # f0/nb0 accumulation), then the rest of wk, then remaining
                # xT blocks, wq, wv
                nc.sync.dma_start(wk_t[:, :, 0:128], wk_r[:, :, 0:128])
                nc.sync.dma_start(xT[:, 0:2, 0:512], xT_r[:, 0:2, 0:512])
                nc.sync.dma_start(xT[:, 2:4, 0:512], xT_r[:, 2:4, 0:512])
                nc.sync.dma_start(xT[:, 4:6, 0:512], xT_r[:, 4:6, 0:512])
                for f in range(1, EC):
                    nc.sync.dma_start(
                        wk_t[:, :, f * 128:(f + 1) * 128],
                        wk_r[:, :, f * 128:(f + 1) * 128],
                    )
                nc.sync.dma_start(xT[:, :, 512:1024], xT_r[:, :, 512:1024])
                nc.sync.dma_start(xT[:, :, 1024:1536], xT_r[:, :, 1024:1536])
                nc.sync.dma_start(xT[:, :, 1536:2048], xT_r[:, :, 1536:2048])
                wq_t = wpp.tile([128, EC, E], F16, tag="w")
                nc.sync.dma_start(wq_t[:], wq_r[:])

                def proj(dst, w_t):
                    # dst = W @ xT   (nb-outer: group nb needs only xT blk nb)
                    for nb in range(NBLK):
                        for f in range(EC):
                            ps = pse.tile([128, 512], F32, tag="ps")
                            for e in range(EC):
                                nc.tensor.matmul(
                                    ps[:],
                                    lhsT=w_t[:, e, f * 128:(f + 1) * 128],
                                    rhs=xT[:, e, nb * 512:(nb + 1) * 512],
                                    start=(e == 0),
                                    stop=(e == EC - 1),
                                )
                            nc.vector.tensor_copy(
                                dst[:, f, nb * 512:(nb + 1) * 512], ps[:]
                            )

                proj(kT, wk_t)
                proj(qT, wq_t)
                # vT stored bf16; wv reuses wk's slot
                wv_t = wpp.tile([128, EC, E], F16, tag="w")
                nc.sync.dma_start(wv_t[:], wv_r[:])
                proj(vT, wv_t)

            # ---------------- attention + vo, software-pipelined ----------
            with tc.tile_pool(name="vo", bufs=1) as vop, \
                 tc.tile_pool(name="wo", bufs=1) as wop, \
                 tc.tile_pool(name="pt", bufs=2) as ptp, \
                 tc.tile_pool(name="pp", bufs=4) as ppp, \
                 tc.tile_pool(name="yt", bufs=12) as ytp, \
                 tc.tile_pool(name="st", bufs=8) as stp:
                vo = vop.tile([128, NT, E], BF16)   # 24; [j-part, jc, f]
                wo_t = wop.tile([128, EC, E], BF16)  # 9
                # dispatched on sync after the inputs; its SBUF region
                # overlaps freed xT, so it lands right after vT-proj ends,
                # during E(b0) — before vo-proj heads the tensor queue
                nc.sync.dma_start(wo_t[:], wo_r[:])

                pt_blks = [None] * NBLK

                def energy_block(ib):
                    pt_blk = ptp.tile([128, NT, 512], BF16)  # 16 x2
                    pt_blks[ib] = pt_blk
                    for t4 in range(4):
                        i = ib * 4 + t4
                        stats = stp.tile([128, 8], F32, tag="stats")
                        e_tiles = []
                        for jb in range(NBLK):
                            pe = pse.tile([128, 512], F32, tag="ps")
                            for d in range(EC):
                                nc.tensor.matmul(
                                    pe[:],
                                    lhsT=qT[:, d, i * 128:(i + 1) * 128],
                                    rhs=kT[:, d, jb * 512:(jb + 1) * 512],
                                    start=(d == 0),
                                    stop=(d == EC - 1),
                                )
                            nc.vector.tensor_reduce(
                                stats[:, jb:jb + 1], pe[:], axis=AX, op=MAX
                            )
                            e_tiles.append(pe)
                        nmax = stp.tile([128, 1], F32, tag="nmax")
                        nc.vector.tensor_reduce(
                            nmax[:], stats[:, 0:4], axis=AX, op=MAX,
                            negate=True,
                        )
                        p_t = ppp.tile([128, N], BF16)  # 4 x4
                        for jb in range(NBLK):
                            nc.scalar.activation(
                                p_t[:, jb * 512:(jb + 1) * 512],
                                e_tiles[jb][:],
                                func=mybir.ActivationFunctionType.Exp,
                                bias=nmax[:],
                                scale=1.0,
                                accum_out=stats[:, 4 + jb:5 + jb],
                            )
                        rs = stp.tile([128, 1], F32, tag="rs")
                        nc.vector.tensor_reduce(
                            rs[:], stats[:, 4:8], axis=AX,
                            op=mybir.AluOpType.add,
                        )
                        rcp = stp.tile([128, 1], F32, tag="rcp")
                        nc.vector.reciprocal(rcp[:], rs[:])
                        nc.vector.tensor_scalar_mul(p_t[:], p_t[:], rcp[:])
                        # ALL transposes on one HWDGE queue, each preceded by
                        # a plain guard DMA: concurrent xbar transposes (even
                        # on different queues) corrupt results — xbar state
                        # is per-core global
                        nc.sync.dma_start(snk_d[i], stats[:])
                        nc.sync.dma_start_transpose(
                            pt_blk[:, :, t4 * 128:(t4 + 1) * 128], p_t[:]
                        )

                yts = [[] for _ in range(NBLK)]

                def pv_block(ib):
                    # yT-block = vo.T @ PT (bf16, 512-wide). The very last
                    # f of the last block is split in halves so the final
                    # store starts ~1.7us earlier. Stores are NOT emitted
                    # here — see store_block.
                    pt_blk = pt_blks[ib]
                    for f in range(EC):
                        last = (ib == NBLK - 1 and f == EC - 1)
                        halves = ((0, 256), (256, 512)) if last else ((0, 512),)
                        for lo, hi in halves:
                            py = psm.tile([128, 512], F32, tag="mm")
                            for jc in range(NT):
                                nc.tensor.matmul(
                                    py[:, :hi - lo],
                                    lhsT=vo[:, jc, f * 128:(f + 1) * 128],
                                    rhs=pt_blk[:, jc, lo:hi],
                                    start=(jc == 0),
                                    stop=(jc == NT - 1),
                                )
                            yt = ytp.tile([128, 512], F16)
                            nc.vector.tensor_copy(
                                yt[:, :hi - lo], py[:, :hi - lo]
                            )
                            yts[ib].append((yt, f, lo, hi))

                def store_block(ib):
                    # yT stores dispatch from the scalar queue. Emitted one
                    # block AFTER the matching pv_block: a store whose yt
                    # evac is pending head-of-line-blocks the next exps on
                    # the scalar queue, which stalls the transposes the PV
                    # matmuls need — a four-engine cycle. One block of
                    # separation guarantees the yt data is ready before any
                    # exp queues behind the store.
                    for yt, f, lo, hi in yts[ib]:
                        nc.scalar.dma_start(
                            yT_r[:, f, ib * 512 + lo:ib * 512 + hi],
                            yt[:, :hi - lo],
                        )

                energy_block(0)

                # vo = vT.T @ Wo.T (bf16); runs while E(b0)'s softmax chain
                # and transposes drain
                for jc in range(NT):
                    for flo, fhi in ((0, 512), (512, 768)):
                        ps = psm.tile([128, 512], F32, tag="mm")
                        for c in range(EC):
                            nc.tensor.matmul(
                                ps[:, :fhi - flo],
                                lhsT=vT[:, c, jc * 128:(jc + 1) * 128],
                                rhs=wo_t[:, c, flo:fhi],
                                start=(c == 0),
                                stop=(c == EC - 1),
                            )
                        nc.vector.tensor_copy(vo[:, jc, flo:fhi], ps[:, :fhi - flo])

                energy_block(1)
                pv_block(0)
                energy_block(2)
                store_block(0)
                pv_block(1)
                energy_block(3)
                store_block(1)
                pv_block(2)
                store_block(2)
                pv_block(3)
                store_block(3)

    nc.finalize()
    return nc


def _get_nc():
    if "nc" not in _CACHE:
        _CACHE["nc"] = _build()
    return _CACHE["nc"]


def kernel(x, Wq, Wk, Wv, Wo, _run_kwargs=None):
    import ml_dtypes
    from concourse.bass_utils import run_bass_kernel_spmd

    x = np.asarray(x, dtype=np.float32)
    # fold the sqrt(head_dim) energy scale into Wq (exact: power of 2)
    wq = np.ascontiguousarray(
        (np.asarray(Wq, dtype=np.float32).T * SCALE).astype(np.float16)
    )
    wk = np.ascontiguousarray(np.asarray(Wk, dtype=np.float32).T.astype(np.float16))
    wv = np.ascontiguousarray(np.asarray(Wv, dtype=np.float32).T.astype(np.float16))
    wo = np.ascontiguousarray(
        np.asarray(Wo, dtype=np.float32).T.astype(ml_dtypes.bfloat16)
    )

    nc = _get_nc()
    in_maps = [
        {
            "xT": np.ascontiguousarray(x[b].T.astype(np.float16)),
            "wq": wq,
            "wk": wk,
            "wv": wv,
            "wo": wo,
        }
        for b in range(B)
    ]
    res = run_bass_kernel_spmd(nc, in_maps, list(range(B)), **(_run_kwargs or {}))
    out = np.stack([res.results[b]["yT"].T for b in range(B)])
    if _run_kwargs:
        _CACHE["last_results"] = res
    return np.ascontiguousarray(out, dtype=np.float32)
